# revision 31
# baseline (speedup 1.0000x reference)
"""DeepSeek-style MoE (E=8, top-6, silu-GLU experts + shared expert) on 8
TRN2 NeuronCores.

Sharding (hardcoded, matches spec sharding_hint):
  - tokens (B*S = 4096) split in 2 halves across core groups {0-3}, {4-7}
  - experts split 4-ways within each group: core handles expert pair
    {2p, 2p+1}, p = core % 4, plus a 256-wide slice of the shared expert's
    FS=1024 dimension.
  - router replicated (with per-core column permutation so that "my"
    experts are always gate columns 0 and 1 -> identical SPMD program).
  - host-side unshard: sum the 4 partial [D, tok] outputs per token half,
    transpose, concat.

Device program (per core, identical SPMD):
  phase R: fp32 router matmul -> softmax -> top-6 mask -> gate [tok, 8]
  phase G: f32r GLU: hT[f, tok] = silu(Wg^T x) * (Wu^T x) * bcast(gate)
  phase D: f32r down-proj: yT[d, tok] = Wd^T hT, accumulated over f in PSUM

Everything except the router runs in float32r (PE "replicated fp32":
bf16-rate matmuls, ~1.5e-4 abs accuracy).
"""

import numpy as np
from contextlib import ExitStack

# ---- model dims (hardcoded from the problem spec) ----
B, S, D, E, F, FS = 2, 2048, 1024, 8, 512, 1024
TOP_K = 6
T = B * S                     # 4096 tokens total
TOK = T // 2                  # 2048 tokens per core (token half)
FL = 2 * F + FS // 4          # 1280 local F: expert0 | expert1 | shared slice
DC = D // 128                 # 8 contraction chunks
FC = FL // 128                # 10 local F chunks (0-3: e0, 4-7: e1, 8-9: shared)
NTH = 2                       # token-half sub-blocks per core
TTH = TOK // NTH              # 1024 tokens per sub-block
TT = TTH // 128               # 8 token tiles per sub-block
TS = TTH // 512               # 2 512-token slices per sub-block

_CACHE = {}


def _build_nc():
    import concourse.bacc as bacc
    import concourse.tile as tile
    from concourse import mybir, masks

    f32 = mybir.dt.float32
    f32r = mybir.dt.float32r
    AF = mybir.ActivationFunctionType
    OP = mybir.AluOpType
    AX = mybir.AxisListType

    nc = bacc.Bacc("TRN2", target_bir_lowering=False, debug=False)

    # all big inputs are host-permuted so that every DMA reads contiguous
    # 4KB+ rows per partition (512B strided segments choke the SDMA)
    # xq[th, g, p, cc, t] = x[c*128+p, th*TTH+t], c = g*4+cc
    xq = nc.dram_tensor("xq", [NTH, 2, 128, 4 * TTH], f32,
                        kind="ExternalInput").ap()
    rw = nc.dram_tensor("rw", [D, E], f32, kind="ExternalInput").ap()
    # block identity: idq[32j+i, i] = 1 — summing combiner for col groups
    idq = nc.dram_tensor("idq", [128, E], f32, kind="ExternalInput").ap()
    # wg[fc, p, c*128+f'] = Wg[c*128+p, fc*128+f']  (ditto wu)
    wg = nc.dram_tensor("wg", [FC, 128, D], f32, kind="ExternalInput").ap()
    wu = nc.dram_tensor("wu", [FC, 128, D], f32, kind="ExternalInput").ap()
    # wd[dc, p, fc*128+d'] = Wd[fc*128+p, dc*128+d']
    wd = nc.dram_tensor("wd", [DC, 128, FL], f32, kind="ExternalInput").ap()
    yT = nc.dram_tensor("yT", [D, TOK], f32, kind="ExternalOutput").ap()

    rw_v = rw.rearrange("(c p) e -> p c e", p=128)     # [128, DC, E]

    with tile.TileContext(nc) as tc:
        with ExitStack() as ctx:
            ep = ctx.enter_context

            cpool = ep(tc.tile_pool(name="consts", bufs=1))
            stage = ep(tc.tile_pool(name="stage", bufs=2))
            xrp = ep(tc.tile_pool(name="xr", bufs=1))
            hp = ep(tc.tile_pool(name="hT", bufs=1))
            wp = ep(tc.tile_pool(name="w", bufs=3))
            wdp = ep(tc.tile_pool(name="wd", bufs=3))
            gp = ep(tc.tile_pool(name="gate", bufs=2))
            gtp = ep(tc.tile_pool(name="gateT", bufs=4))
            bcp = ep(tc.tile_pool(name="bcast", bufs=4))
            sp = ep(tc.tile_pool(name="glu", bufs=3))
            yp = ep(tc.tile_pool(name="yout", bufs=3))

            # 8 PSUM banks total: misc(2) + g(2) + u(2) + y(2)
            mps = ep(tc.tile_pool(name="mps", bufs=2, space="PSUM"))
            gps_p = ep(tc.tile_pool(name="gps", bufs=2, space="PSUM"))
            ups_p = ep(tc.tile_pool(name="ups", bufs=2, space="PSUM"))
            yps = ep(tc.tile_pool(name="yps", bufs=2, space="PSUM"))

            identity = cpool.tile([128, 128], f32)
            masks.make_identity(nc, identity[:])
            identity_r = cpool.tile([128, 128], f32r)
            nc.vector.tensor_copy(identity_r[:], identity[:])
            ones_f = cpool.tile([1, 128], f32)
            nc.vector.memset(ones_f[:], 1.0)
            ones = cpool.tile([1, 128], f32r)
            nc.vector.tensor_copy(ones[:], ones_f[:])
            rw_sb = cpool.tile([128, DC, E], f32)
            nc.sync.dma_start(rw_sb[:], rw_v)
            idq_sb = cpool.tile([128, E], f32)
            nc.sync.dma_start(idq_sb[:], idq)

            for th in range(NTH):
                t0 = th * TTH  # token offset of this sub-block within core

                # ---- stage x per chunk; transposed router scores zT[e, tok].
                # 4-way col-tiled over PE column groups: chunk pair (j, j+4)
                # accumulates into partitions [32j:32j+8]; the 4 group
                # partials are summed by the accumulating transposes below.
                # DMA order c = j, j+4, ... so each group's pair is
                # consecutive (avoids interleaved PSUM accumulation groups,
                # whose start=True clears the whole bank's has_written bits).
                # Cast each chunk to f32r as it lands. ----
                xr = xrp.tile([128, DC, TTH], f32r, tag="xr")

                wpref = {}

                def prefetch_chunk(fc):
                    wg_t = wp.tile([128, DC, 128], f32r, tag="wg",
                                   name=f"wg{th}_{fc}")
                    nc.scalar.dma_start(
                        wg_t[:],
                        wg[fc].rearrange("p (c f) -> p c f", c=DC).bitcast(f32r),
                    )
                    wu_t = wp.tile([128, DC, 128], f32r, tag="wu",
                                   name=f"wu{th}_{fc}")
                    nc.scalar.dma_start(
                        wu_t[:],
                        wu[fc].rearrange("p (c f) -> p c f", c=DC).bitcast(f32r),
                    )
                    wpref[fc] = (wg_t, wu_t)

                # prefetch the first (shared) GLU chunks before the router so
                # their DMA issues aren't queued behind the softmax on ACT
                prefetch_chunk(8)
                prefetch_chunk(9)

                zt = [mps.tile([128, 512], f32, tag="m", name=f"zt{th}_{ts}")
                      for ts in range(TS)]
                for g in range(2):
                    xs_g = stage.tile([128, 4, TTH], f32, tag="xs",
                                      name=f"xs{th}_{g}")
                    xq_v = xq[th, g].rearrange("p (cc t) -> p cc t", cc=4)
                    # two 1MB DMAs so the first col-group pair can start
                    # as soon as the first half lands
                    nc.sync.dma_start(xs_g[:, 0:2, :], xq_v[:, 0:2, :])
                    nc.sync.dma_start(xs_g[:, 2:4, :], xq_v[:, 2:4, :])
                    for jj in range(2):
                        j = g * 2 + jj
                        for r in range(2):
                            cl = jj * 2 + r        # chunk within this half
                            c = g * 4 + cl         # global chunk
                            for ts in range(TS):
                                nc.tensor.matmul(
                                    zt[ts][32 * j:32 * j + 8, :],
                                    rw_sb[:, c, :],
                                    xs_g[:, cl, ts * 512:(ts + 1) * 512],
                                    start=(r == 0),
                                    stop=(r == 1),
                                    tile_position=(0, 32 * j),
                                )
                    nc.vector.tensor_copy(
                        xr[:, g * 4:(g + 1) * 4, :], xs_g[:]
                    )
                zT_sb = gp.tile([128, TTH], f32, tag="zT_sb")
                for ts in range(TS):
                    nc.vector.tensor_copy(
                        zT_sb[:, ts * 512:(ts + 1) * 512], zt[ts][:]
                    )

                # combine + transpose the 4 col-group partials back to
                # token-major z[tok, e] in one matmul per token tile:
                # z[:, t, :] = zT_slice.T @ idq  (idq sums the groups)
                z_ps = mps.tile([128, TT, E], f32, tag="m")
                for t in range(TT):
                    nc.tensor.matmul(
                        z_ps[:, t, :],
                        zT_sb[:, t * 128:(t + 1) * 128],
                        idq_sb[:],
                        start=True, stop=True,
                    )

                # ---- softmax (no max-sub; |z| is small) + top-6 mask ----
                z_sb = gp.tile([128, TT, E], f32, tag="z_sb")
                nc.vector.tensor_copy(z_sb[:], z_ps[:])
                e_sb = gp.tile([128, TT, E], f32, tag="e_sb")
                nc.scalar.activation(e_sb[:], z_sb[:], AF.Exp)
                esum = gp.tile([128, TT], f32, tag="esum")
                nc.vector.tensor_reduce(esum[:], e_sb[:], AX.X, OP.add)
                rcp = gp.tile([128, TT], f32, tag="rcp")
                nc.vector.reciprocal(rcp[:], esum[:])
                m1 = gp.tile([128, TT], f32, tag="m1")
                nc.vector.tensor_reduce(m1[:], z_sb[:], AX.X, OP.min)
                eq = gp.tile([128, TT, E], f32, tag="eq")
                for t in range(TT):
                    nc.vector.tensor_scalar(
                        eq[:, t, :], z_sb[:, t, :], m1[:, t:t + 1], None, OP.is_equal
                    )
                zb = gp.tile([128, TT, E], f32, tag="zb")
                nc.vector.scalar_tensor_tensor(
                    zb[:], eq[:], 1e30, z_sb[:], op0=OP.mult, op1=OP.add
                )
                m2 = gp.tile([128, TT], f32, tag="m2")
                nc.vector.tensor_reduce(m2[:], zb[:], AX.X, OP.min)
                gate = gp.tile([128, TT, E], f32r, tag="gate")
                for t in range(TT):
                    # keep = z > m2 ? 1 : 0 ; gate = p * keep, p = e * rcp
                    nc.vector.tensor_scalar(
                        gate[:, t, :], z_sb[:, t, :], m2[:, t:t + 1], None, OP.is_gt
                    )
                    nc.vector.tensor_scalar(
                        e_sb[:, t, :], e_sb[:, t, :], rcp[:, t:t + 1], None, OP.mult
                    )
                nc.vector.tensor_mul(gate[:], gate[:], e_sb[:])

                # ---- GLU: hT[f, tok] ----
                # shared-expert chunks (8, 9) are emitted FIRST: they need no
                # gate, so the PE keeps streaming while the softmax chain
                # (DVE/ACT) finishes; the gate-prep PE ops (transposes +
                # broadcast matmuls) are emitted between them and the gated
                # expert chunks.
                hT = hp.tile([128, FC, TTH], f32r, tag="hT")
                bcast = {}

                def glu_chunk(fc):
                    wg_t, wu_t = wpref.pop(fc)
                    for ts in range(TS):
                        sl = slice(ts * 512, (ts + 1) * 512)
                        g_ps = gps_p.tile([128, 512], f32, tag="g",
                                          name=f"g{th}_{fc}_{ts}")
                        for c in range(DC):
                            nc.tensor.matmul(
                                g_ps[:], wg_t[:, c, :], xr[:, c, sl],
                                start=(c == 0), stop=(c == DC - 1),
                            )
                        u_ps = ups_p.tile([128, 512], f32, tag="u",
                                          name=f"u{th}_{fc}_{ts}")
                        for c in range(DC):
                            nc.tensor.matmul(
                                u_ps[:], wu_t[:, c, :], xr[:, c, sl],
                                start=(c == 0), stop=(c == DC - 1),
                            )
                        sg = sp.tile([128, 512], f32r, tag="sg",
                                     name=f"sg{th}_{fc}_{ts}")
                        nc.scalar.activation(sg[:], g_ps[:], AF.Silu)
                        if fc < 8:
                            ug = sp.tile([128, 512], f32, tag="ug",
                                         name=f"ug{th}_{fc}_{ts}")
                            nc.vector.tensor_mul(
                                ug[:], bcast[(fc // 4, ts)][:], u_ps[:]
                            )
                            nc.vector.tensor_mul(hT[:, fc, sl], sg[:], ug[:])
                        else:
                            nc.vector.tensor_mul(hT[:, fc, sl], sg[:], u_ps[:])

                prefetch_chunk(0)
                glu_chunk(8)
                prefetch_chunk(1)
                glu_chunk(9)

                # ---- gate columns 0,1 -> broadcast [128, tok] tiles ----
                for e in range(2):
                    for ts in range(TS):
                        gt_ps = mps.tile([1, 512], f32r, tag="m",
                                         name=f"gt{th}_{e}_{ts}")
                        for tq in range(4):
                            t = ts * 4 + tq
                            nc.tensor.transpose(
                                gt_ps[0:1, tq * 128:(tq + 1) * 128],
                                gate[:, t, e:e + 1],
                                identity_r[:],
                            )
                        gt_sb = gtp.tile([1, 512], f32r, tag="gt_sb",
                                         name=f"gtsb{th}_{e}_{ts}")
                        nc.vector.tensor_copy(gt_sb[:], gt_ps[:])
                        bc_ps = mps.tile([128, 512], f32, tag="m",
                                         name=f"bc{th}_{e}_{ts}")
                        nc.tensor.matmul(
                            bc_ps[:], ones[0:1, :], gt_sb[0:1, :], start=True, stop=True
                        )
                        bc_sb = bcp.tile([128, 512], f32, tag="bc_sb",
                                         name=f"bcsb{th}_{e}_{ts}")
                        nc.scalar.copy(bc_sb[:], bc_ps[:])
                        bcast[(e, ts)] = bc_sb

                for fc in range(8):
                    if fc + 2 < 8:
                        prefetch_chunk(fc + 2)
                    glu_chunk(fc)

                # ---- down-proj: yT[d, tok] ----
                for dc in range(DC):
                    wd_t = wdp.tile([128, FC, 128], f32r, tag="wd")
                    nc.sync.dma_start(
                        wd_t[:],
                        wd[dc].rearrange("p (f d) -> p f d", f=FC).bitcast(f32r),
                    )
                    for ts in range(TS):
                        sl = slice(ts * 512, (ts + 1) * 512)
                        y_ps = yps.tile([128, 512], f32, tag="y")
                        for fc in range(FC):
                            nc.tensor.matmul(
                                y_ps[:], wd_t[:, fc, :], hT[:, fc, sl],
                                start=(fc == 0), stop=(fc == FC - 1),
                            )
                        y_sb = yp.tile([128, 512], f32, tag="y_sb")
                        nc.vector.tensor_copy(y_sb[:], y_ps[:])
                        nc.sync.dma_start(
                            yT[dc * 128:(dc + 1) * 128,
                               t0 + ts * 512:t0 + (ts + 1) * 512],
                            y_sb[:],
                        )

    nc.compile()
    return nc


def _get_nc():
    if "nc" not in _CACHE:
        _CACHE["nc"] = _build_nc()
    return _CACHE["nc"]


def _shard_inputs(hidden_states, router_w, w_gate, w_up, w_down,
                  ws_gate, ws_up, ws_down):
    x = np.asarray(hidden_states, np.float32).reshape(T, D)
    idq = np.zeros((128, E), np.float32)
    for j in range(4):
        for i in range(E):
            idq[32 * j + i, i] = 1.0
    in_maps = []
    for c in range(8):
        th, p = divmod(c, 4)
        e0, e1 = 2 * p, 2 * p + 1
        perm = [e0, e1] + [e for e in range(E) if e not in (e0, e1)]
        fs = slice(p * (FS // 4), (p + 1) * (FS // 4))
        xT_c = x[th * TOK:(th + 1) * TOK, :].T          # [D, TOK]
        # xq[th2, g, p, cc*TTH + t] = xT_c[(g*4+cc)*128 + p, th2*TTH + t]
        xq_c = np.ascontiguousarray(
            xT_c.reshape(2, 4, 128, NTH, TTH).transpose(3, 0, 2, 1, 4)
            .reshape(NTH, 2, 128, 4 * TTH))
        rw_c = np.ascontiguousarray(np.asarray(router_w, np.float32)[:, perm])
        wg_full = np.concatenate(
            [w_gate[e0], w_gate[e1], ws_gate[:, fs]], axis=1, dtype=np.float32)
        wu_full = np.concatenate(
            [w_up[e0], w_up[e1], ws_up[:, fs]], axis=1, dtype=np.float32)
        wd_full = np.concatenate(
            [w_down[e0], w_down[e1], ws_down[fs, :]], axis=0, dtype=np.float32)
        # wg[fc, p, c*128+f'] = wg_full[c*128+p, fc*128+f']
        wg_c = np.ascontiguousarray(
            wg_full.reshape(DC, 128, FC, 128).transpose(2, 1, 0, 3)
            .reshape(FC, 128, D))
        wu_c = np.ascontiguousarray(
            wu_full.reshape(DC, 128, FC, 128).transpose(2, 1, 0, 3)
            .reshape(FC, 128, D))
        # wd[dc, p, fc*128+d'] = wd_full[fc*128+p, dc*128+d']
        wd_c = np.ascontiguousarray(
            wd_full.reshape(FC, 128, DC, 128).transpose(2, 1, 0, 3)
            .reshape(DC, 128, FL))
        in_maps.append({"xq": xq_c, "rw": rw_c, "wg": wg_c, "wu": wu_c,
                        "wd": wd_c, "idq": idq})
    return in_maps


def _run(in_maps, **kwargs):
    from concourse import bass_utils
    nc = _get_nc()
    return bass_utils.run_bass_kernel_spmd(
        nc, in_maps, core_ids=list(range(8)), **kwargs
    )


def _unshard(results):
    parts = [r["yT"] for r in results]
    y0 = parts[0] + parts[1] + parts[2] + parts[3]   # [D, TOK]
    y1 = parts[4] + parts[5] + parts[6] + parts[7]
    y = np.concatenate([y0.T, y1.T], axis=0)         # [T, D]
    return np.ascontiguousarray(y.reshape(B, S, D).astype(np.float32))


def kernel(**inputs):
    in_maps = _shard_inputs(**inputs)
    res = _run(in_maps)
    return _unshard(res.results)


def kernel_profiled(**inputs):
    """Like kernel(), but with NTFF tracing; returns (y, BassKernelResults)."""
    in_maps = _shard_inputs(**inputs)
    res = _run(in_maps, trace=True)
    return _unshard(res.results), res
